# revision 11
# baseline (speedup 1.0000x reference)
"""Trainium2 Bass kernel for nn_CausalSelfAttention (sparse_attention).

Tensor-parallel over heads across 8 NeuronCores: core h owns head h.
Per core: qkv projection (fp32r matmuls), RMSNorm + RoPE, causal
attention without max-subtraction (RMSNorm bounds |score*0.12| <= 15.36),
c_proj partial in transposed layout, and a per-t-block on-device
ReduceScatter(add) that combines the per-head c_proj partials while the
next block computes. Host side only reshapes/transposes (shard/unshard).
"""
import numpy as np

import concourse.bass as bass
import concourse.tile as tile
from concourse import bacc, mybir
from concourse.alu_op_type import AluOpType
from concourse.bass_utils import run_bass_kernel_spmd

DIM = 1024
NUM_HEADS = 8
HEAD_DIM = 128
T = 2048
ATTN_SCALE = 0.12
NCORES = 8

P = 128
NT = T // P            # 16 t-chunks of 128
NB = T // 512          # 4 t-blocks of 512
ND = DIM // P          # 8 D-chunks

F32 = mybir.dt.float32
F32R = mybir.dt.float32r
AF = mybir.ActivationFunctionType


def _rope_tables():
    n = HEAD_DIM // 4
    freq = (1.0 / 1024.0) ** np.linspace(0.0, 1.0, n, dtype=np.float32)
    freq = np.concatenate([freq, np.zeros((n,), np.float32)])  # [64]
    t = np.arange(T, dtype=np.float32)
    theta = t[:, None] * freq[None, :]
    return np.cos(theta).astype(np.float32), np.sin(theta).astype(np.float32)


def build_kernel(with_rs=True):
    nc = bacc.Bacc("TRN2", target_bir_lowering=False, debug=False,
                   num_devices=NCORES)

    # ---- I/O ----
    xT = nc.dram_tensor("xT", [DIM, T], F32R, kind="ExternalInput").ap()
    wqkvT = nc.dram_tensor("wqkvT", [DIM, 3 * P], F32R, kind="ExternalInput").ap()
    cpwT = nc.dram_tensor("cpwT", [P, DIM], F32R, kind="ExternalInput").ap()
    ve_s = nc.dram_tensor("ve_s", [T, P], F32, kind="ExternalInput").ap()
    cos_t = nc.dram_tensor("cos_t", [T, 64], F32, kind="ExternalInput").ap()
    sin_t = nc.dram_tensor("sin_t", [T, 64], F32, kind="ExternalInput").ap()
    ident = nc.dram_tensor("ident", [P, P], F32R, kind="ExternalInput").ap()
    tri = nc.dram_tensor("tri", [P, P], F32, kind="ExternalInput").ap()  # s<=t
    out_s = nc.dram_tensor("out_s", [P, T], F32, kind="ExternalOutput").ap()

    # per-block internal DRAM for the pipelined collective
    partials = [nc.dram_tensor(f"partial{i}", [DIM, 512], F32) for i in range(NB)]
    rs_outs = [nc.dram_tensor(f"rs_out{i}", [P, 512], F32) for i in range(NB)]

    eps = float(np.finfo(np.float32).eps)
    xT_r = xT.rearrange("(o p) t -> p o t", p=P)

    with tile.TileContext(nc) as tc:
        with (
            tc.tile_pool(name="big", bufs=1) as big,
            tc.tile_pool(name="work", bufs=3) as work,
            tc.tile_pool(name="rope", bufs=1) as rope,
            tc.tile_pool(name="small", bufs=4) as small,
            tc.tile_pool(name="ps_mm", bufs=3, space="PSUM") as ps_mm,
            tc.tile_pool(name="ps_acc", bufs=2, space="PSUM") as ps_acc,
            tc.tile_pool(name="ps_tr", bufs=2, space="PSUM") as ps_tr,
        ):
            # ---- loads: xT block 0 first (sync), weights on scalar ring,
            #      the rest on gpsimd so everything streams in parallel ----
            xT_sb = big.tile([P, ND, T], F32R)
            nc.sync.dma_start(xT_sb[:, :, 0:512], xT_r[:, :, 0:512])
            w_sb = big.tile([P, ND, 3 * P], F32R)
            nc.scalar.dma_start(w_sb, wqkvT.rearrange("(o p) f -> p o f", p=P))
            for b in range(1, NB):
                nc.sync.dma_start(xT_sb[:, :, b * 512:(b + 1) * 512],
                                  xT_r[:, :, b * 512:(b + 1) * 512])
            cpw_sb = big.tile([P, DIM], F32R)
            nc.scalar.dma_start(cpw_sb, cpwT[:, :])
            id_sb = big.tile([P, P], F32R)
            nc.gpsimd.dma_start(id_sb, ident[:, :])
            tri_sb = big.tile([P, P], F32)
            nc.gpsimd.dma_start(tri_sb, tri[:, :])
            cos_sb = big.tile([P, NT, 64], F32)
            nc.gpsimd.dma_start(cos_sb, cos_t.rearrange("(c p) j -> p c j", p=P))
            sin_sb = big.tile([P, NT, 64], F32)
            nc.gpsimd.dma_start(sin_sb, sin_t.rearrange("(c p) j -> p c j", p=P))
            ve_sb = big.tile([P, NT, P], F32)
            nc.gpsimd.dma_start(ve_sb, ve_s.rearrange("(c p) d -> p c d", p=P))
            ones_f = small.tile([P, 1], F32)
            nc.vector.memset(ones_f, 1.0)
            ones_r = big.tile([P, 1], F32R)
            nc.vector.tensor_copy(ones_r, ones_f)
            ones1_f = big.tile([1, P], F32)
            nc.vector.memset(ones1_f, 1.0)
            eps_sb = big.tile([P, 1], F32)
            nc.vector.memset(eps_sb, eps)
            zeros_sb = big.tile([P, 384], F32)
            nc.vector.memset(zeros_sb, 0.0)

            # ---- phase 1 (per 512-block): proj + RMS + RoPE + transposes ----
            q_r = big.tile([P, NT, P], F32R)
            k_r = big.tile([P, NT, P], F32R)
            v_r = big.tile([P, NT, P], F32R)
            qT_sb = big.tile([P, T], F32R)
            kT_sb = big.tile([P, T], F32R)

            for b in range(NB):
                for c in range(4 * b, 4 * b + 4):
                    qkv_full = ps_mm.tile([P, 512], F32, tag="mm")
                    qkv_ps = qkv_full[:, :3 * P]
                    for o in range(ND):
                        nc.tensor.matmul(
                            qkv_ps,
                            xT_sb[:, o, c * P:(c + 1) * P],
                            w_sb[:, o, :],
                            start=(o == 0), stop=(o == ND - 1),
                        )
                    # sum of squares for q,k via ACT square with accumulate
                    sq_scr = work.tile([P, 2 * P], F32, tag="sqscr")
                    ssq = small.tile([P, 2], F32, tag="ssq")
                    nc.scalar.activation(sq_scr[:, :P], qkv_ps[:, :P], AF.Square,
                                         accum_out=ssq[:, 0:1])
                    nc.scalar.activation(sq_scr[:, P:], qkv_ps[:, P:2 * P],
                                         AF.Square, accum_out=ssq[:, 1:2])
                    rqk = small.tile([P, 2], F32, tag="rqk")
                    nc.scalar.activation(rqk, ssq, AF.Sqrt, bias=eps_sb,
                                         scale=1.0 / HEAD_DIM)
                    nc.vector.reciprocal(rqk, rqk)
                    nc.vector.tensor_scalar_mul(q_r[:, c, :], qkv_ps[:, :P],
                                                rqk[:, 0:1])
                    nc.vector.tensor_scalar_mul(k_r[:, c, :], qkv_ps[:, P:2 * P],
                                                rqk[:, 1:2])
                    nc.vector.tensor_tensor(v_r[:, c, :], qkv_ps[:, 2 * P:],
                                            ve_sb[:, c, :], AluOpType.add)

                # RoPE for this block's 4 chunks (in place on q_r/k_r)
                csl = slice(4 * b, 4 * b + 4)
                for t_r in (q_r, k_r):
                    t1 = rope.tile([P, 4, 32], F32, tag="t1")
                    t2 = rope.tile([P, 4, 32], F32, tag="t2")
                    t4 = rope.tile([P, 4, 32], F32, tag="t4")
                    t5 = rope.tile([P, 4, 32], F32, tag="t5")
                    x1 = t_r[:, csl, 0:32]
                    x2 = t_r[:, csl, 64:96]
                    ca = cos_sb[:, csl, 0:32]
                    sa = sin_sb[:, csl, 0:32]
                    nc.vector.tensor_tensor(t1, x1, ca, AluOpType.mult)
                    nc.vector.tensor_tensor(t2, x2, sa, AluOpType.mult)
                    nc.vector.tensor_tensor(t4, x1, sa, AluOpType.mult)
                    nc.vector.tensor_tensor(t5, x2, ca, AluOpType.mult)
                    nc.vector.tensor_tensor(x1, t1, t2, AluOpType.add)
                    nc.vector.tensor_tensor(x2, t5, t4, AluOpType.subtract)

                # transposes for this block (evac split ACT/DVE)
                for c in range(4 * b, 4 * b + 4):
                    for j, (src, dstT) in enumerate(((q_r, qT_sb), (k_r, kT_sb))):
                        tp_full = ps_tr.tile([P, 512], F32R, tag="tp",
                                             name="tp_full")
                        tp = tp_full[:, :P]
                        nc.tensor.transpose(tp, src[:, c, :], id_sb)
                        if j == 0:
                            nc.scalar.copy(dstT[:, c * P:(c + 1) * P], tp)
                        else:
                            nc.vector.tensor_copy(dstT[:, c * P:(c + 1) * P], tp)

            # ---- phase 2: attention + pipelined c_proj + ReduceScatter ----
            yT_sb = big.tile([P, T], F32R)
            for i in range(NB):
                tsl = slice(i * 512, (i + 1) * 512)
                n_s = 4 * i + 4
                yT_ps = ps_acc.tile([P, 512], F32, tag="yT")
                den_ps = ps_tr.tile([1, 512], F32, tag="tp")
                for s in range(n_s):
                    lo = s - 4 * i            # staircase chunk idx when >= 0
                    m0 = 0 if lo < 0 else min(lo, 2) * P   # matmul col start
                    e0 = 0 if lo < 0 else lo * P           # exp col start
                    sc_ps = ps_mm.tile([P, 512], F32, tag="mm")
                    nc.tensor.matmul(sc_ps[:, m0:], kT_sb[:, s * P:(s + 1) * P],
                                     qT_sb[:, i * 512 + m0:(i + 1) * 512],
                                     start=True, stop=True)
                    pT = work.tile([P, 512], F32R, tag="pT")
                    nc.scalar.activation(pT[:, e0:], sc_ps[:, e0:], AF.Exp,
                                         scale=ATTN_SCALE)
                    if lo > 0:
                        nc.vector.tensor_copy(pT[:, :e0], zeros_sb[:, :e0])
                    if lo >= 0:
                        nc.vector.tensor_tensor(
                            pT[:, e0:e0 + P], pT[:, e0:e0 + P], tri_sb,
                            AluOpType.mult)
                    nc.tensor.matmul(yT_ps[:, m0:], v_r[:, s, :], pT[:, m0:],
                                     start=(s == 0), stop=(s == n_s - 1))
                    nc.tensor.matmul(den_ps[:, m0:], ones_r, pT[:, m0:],
                                     start=(s == 0), stop=(s == n_s - 1))
                # denominator: broadcast (fp32 K=1 matmul), recip, scale
                den_sb = small.tile([1, 512], F32, tag="densb")
                nc.vector.tensor_copy(den_sb, den_ps)
                bc_ps = ps_tr.tile([P, 512], F32, tag="tp")
                nc.tensor.matmul(bc_ps, ones1_f, den_sb, start=True, stop=True)
                rec_sb = work.tile([P, 512], F32, tag="rec")
                nc.vector.reciprocal(rec_sb, bc_ps)
                nc.vector.tensor_tensor(yT_sb[:, tsl], yT_ps, rec_sb,
                                        AluOpType.mult)

                # c_proj for this block; partial -> DRAM -> ReduceScatter
                for o in range(ND):
                    cp_ps = ps_mm.tile([P, 512], F32, tag="mm")
                    nc.tensor.matmul(cp_ps, cpw_sb[:, o * P:(o + 1) * P],
                                     yT_sb[:, tsl], start=True, stop=True)
                    ev = work.tile([P, 512], F32, tag="ev")
                    nc.vector.tensor_copy(ev, cp_ps)
                    nc.sync.dma_start(partials[i][o * P:(o + 1) * P, :], ev)
                if with_rs:
                    nc.gpsimd.collective_compute(
                        "ReduceScatter",
                        mybir.AluOpType.add,
                        replica_groups=[list(range(NCORES))],
                        ins=[partials[i][:].opt()],
                        outs=[rs_outs[i][:].opt()],
                    )
                    res = work.tile([P, 512], F32, tag="res")
                    nc.sync.dma_start(res, rs_outs[i][:, :])
                    nc.sync.dma_start(out_s[:, tsl], res)

    nc.compile()
    return nc


_NC_CACHE = None
_LAST_RES = None


def kernel(x, ve, lambdas, qkv_w, c_proj_w):
    global _NC_CACHE, _LAST_RES
    if _NC_CACHE is None:
        _NC_CACHE = build_kernel()
    nc = _NC_CACHE

    x = np.asarray(x, dtype=np.float32)
    ve = np.asarray(ve, dtype=np.float32)
    lambdas = np.asarray(lambdas, dtype=np.float32)
    qkv_w = np.asarray(qkv_w, dtype=np.float32)
    c_proj_w = np.asarray(c_proj_w, dtype=np.float32)

    lam0, lam1 = float(lambdas[0]), float(lambdas[1])
    xT = np.ascontiguousarray(x[0].T)                       # [1024, 2048]
    cos, sin = _rope_tables()
    identity = np.eye(P, dtype=np.float32)
    tri = (np.arange(P)[:, None] <= np.arange(P)[None, :]).astype(np.float32)

    in_maps = []
    for h in range(NCORES):
        hs = slice(h * P, (h + 1) * P)
        wq = qkv_w[0, hs, :]
        wk = qkv_w[1, hs, :]
        wv = qkv_w[2, hs, :] * lam0
        wqkvT = np.ascontiguousarray(
            np.concatenate([wq, wk, wv], axis=0).T)          # [1024, 384]
        cpwT_h = np.ascontiguousarray(c_proj_w[:, hs].T)     # [128, 1024]
        ve_h = np.ascontiguousarray(ve[0][:, hs] * lam1)     # [2048, 128]
        in_maps.append({
            "xT": xT,
            "wqkvT": wqkvT,
            "cpwT": cpwT_h,
            "ve_s": ve_h,
            "cos_t": cos,
            "sin_t": sin,
            "ident": identity,
            "tri": tri,
        })

    res = run_bass_kernel_spmd(nc, in_maps, core_ids=list(range(NCORES)))
    _LAST_RES = res
    # core h's out_s holds rows [128h, 128h+128) of outT for every t block
    outT = np.concatenate([res.results[h]["out_s"] for h in range(NCORES)],
                          axis=0)                            # [1024, 2048]
    return np.ascontiguousarray(outT.T)[None].astype(np.float32)


# revision 33
# speedup vs baseline: 17529.7968x; 17529.7968x over previous
"""Trainium2 Bass kernel for nn_CausalSelfAttention (sparse_attention).

Tensor-parallel over heads across 8 NeuronCores: core h owns head h.
Per core: qkv projection (fp32r matmuls), RMSNorm + RoPE, causal
attention without max-subtraction (RMSNorm bounds |score*0.12| <= 15.36),
c_proj partial in transposed layout, and a per-t-block on-device
ReduceScatter(add) that combines the per-head c_proj partials while the
next block computes. Host side only reshapes/transposes (shard/unshard).
"""
import numpy as np

import concourse.bass as bass
import concourse.tile as tile
from concourse import bacc, mybir
from concourse.alu_op_type import AluOpType
from concourse.bass_utils import run_bass_kernel_spmd

DIM = 1024
NUM_HEADS = 8
HEAD_DIM = 128
T = 2048
ATTN_SCALE = 0.12
NCORES = 8

P = 128
NT = T // P            # 16 t-chunks of 128
NB = T // 512          # 4 t-blocks of 512
ND = DIM // P          # 8 D-chunks

F32 = mybir.dt.float32
F32R = mybir.dt.float32r
AF = mybir.ActivationFunctionType


def _rope_tables():
    n = HEAD_DIM // 4
    freq = (1.0 / 1024.0) ** np.linspace(0.0, 1.0, n, dtype=np.float32)
    freq = np.concatenate([freq, np.zeros((n,), np.float32)])  # [64]
    t = np.arange(T, dtype=np.float32)
    theta = t[:, None] * freq[None, :]
    return np.cos(theta).astype(np.float32), np.sin(theta).astype(np.float32)


def build_kernel(with_rs=True):
    nc = bacc.Bacc("TRN2", target_bir_lowering=False, debug=False,
                   num_devices=NCORES)

    # ---- I/O ----
    xT = nc.dram_tensor("xT", [DIM, T], F32R, kind="ExternalInput").ap()
    wqkvT = nc.dram_tensor("wqkvT", [DIM, 3 * P], F32R, kind="ExternalInput").ap()
    cpwT = nc.dram_tensor("cpwT", [P, DIM], F32R, kind="ExternalInput").ap()
    ve_s = nc.dram_tensor("ve_s", [T, P], F32, kind="ExternalInput").ap()
    cos_t = nc.dram_tensor("cos_t", [T, 64], F32, kind="ExternalInput").ap()
    sin_t = nc.dram_tensor("sin_t", [T, 64], F32, kind="ExternalInput").ap()
    ident = nc.dram_tensor("ident", [P, P], F32R, kind="ExternalInput").ap()
    tri = nc.dram_tensor("tri", [P, P], F32, kind="ExternalInput").ap()  # s<=t
    out_s = nc.dram_tensor("out_s", [P, T], F32, kind="ExternalOutput").ap()

    # per-block internal DRAM for the pipelined collective
    partials = [nc.dram_tensor(f"partial{i}", [DIM, 512], F32) for i in range(NB)]
    rs_outs = [nc.dram_tensor(f"rs_out{i}", [P, 512], F32) for i in range(NB)]

    eps = float(np.finfo(np.float32).eps)
    xT_r = xT.rearrange("(o p) t -> p o t", p=P)

    with tile.TileContext(nc) as tc:
        with (
            tc.tile_pool(name="big", bufs=1) as big,
            tc.tile_pool(name="work", bufs=5) as work,
            tc.tile_pool(name="rope", bufs=1) as rope,
            tc.tile_pool(name="small", bufs=4) as small,
            tc.tile_pool(name="ps_mm", bufs=4, space="PSUM") as ps_mm,
            tc.tile_pool(name="ps_acc", bufs=1, space="PSUM") as ps_acc,
            tc.tile_pool(name="ps_tr", bufs=2, space="PSUM") as ps_tr,
            tc.tile_pool(name="ps_den", bufs=1, space="PSUM") as ps_den,
        ):
            # ---- loads: xT block 0 first (sync), weights on scalar ring,
            #      the rest on gpsimd so everything streams in parallel ----
            xT_sb = big.tile([P, ND, T],
                             mybir.dt.bfloat16 if bf16_x else F32R)
            w_sb = big.tile([P, ND, 3 * P], F32R)
            nc.gpsimd.dma_start(w_sb, wqkvT.rearrange("(o p) f -> p o f", p=P))
            for c in range(NT):
                eng = nc.sync if c % 2 == 0 else nc.scalar
                eng.dma_start(xT_sb[:, :, c * P:(c + 1) * P],
                              xT_r[:, :, c * P:(c + 1) * P])
            cpw_sb = big.tile([P, DIM], F32R)
            nc.gpsimd.dma_start(cpw_sb, cpwT[:, :])
            id_sb = big.tile([P, P], F32R)
            nc.gpsimd.dma_start(id_sb, ident[:, :])
            tri_sb = big.tile([P, P], F32)
            nc.gpsimd.dma_start(tri_sb, tri[:, :])
            cos_sb = big.tile([P, NT, 64], F32)
            nc.gpsimd.dma_start(cos_sb, cos_t.rearrange("(c p) j -> p c j", p=P))
            sin_sb = big.tile([P, NT, 64], F32)
            nc.gpsimd.dma_start(sin_sb, sin_t.rearrange("(c p) j -> p c j", p=P))
            ve_sb = big.tile([P, NT, P], F32)
            nc.gpsimd.dma_start(ve_sb, ve_s.rearrange("(c p) d -> p c d", p=P))
            ones_f = small.tile([P, 1], F32)
            nc.vector.memset(ones_f, 1.0)
            ones_r = big.tile([P, 1], F32R)
            nc.vector.tensor_copy(ones_r, ones_f)
            eps_sb = big.tile([P, 1], F32)
            nc.vector.memset(eps_sb, eps)
            zeros_sb = big.tile([P, 384], F32)
            nc.vector.memset(zeros_sb, 0.0)

            # ---- phase 1 (per 512-block): proj + RMS + RoPE + transposes ----
            q_r = big.tile([P, NT, P], F32R)
            k_r = big.tile([P, NT, P], F32R)
            v_r = big.tile([P, NT, P], F32R)
            qT_sb = big.tile([P, T], F32R)
            kT_sb = big.tile([P, T], F32R)

            def _phase1_block(b):
                for c in range(4 * b, 4 * b + 4):
                    qkv_full = ps_mm.tile([P, 512], F32, tag="mm", name="qkv_full")
                    qkv_ps = qkv_full[:, :3 * P]
                    for o in range(ND):
                        nc.tensor.matmul(
                            qkv_ps,
                            xT_sb[:, o, c * P:(c + 1) * P],
                            w_sb[:, o, :],
                            start=(o == 0), stop=(o == ND - 1),
                        )
                    # sum of squares for q,k via ACT square with accumulate
                    sq_scr = work.tile([P, 2 * P], F32, tag="sqscr", name="sq_scr")
                    ssq = small.tile([P, 2], F32, tag="ssq", name="ssq")
                    nc.scalar.activation(sq_scr[:, :P], qkv_ps[:, :P], AF.Square,
                                         accum_out=ssq[:, 0:1])
                    nc.scalar.activation(sq_scr[:, P:], qkv_ps[:, P:2 * P],
                                         AF.Square, accum_out=ssq[:, 1:2])
                    rqk = small.tile([P, 2], F32, tag="rqk", name="rqk")
                    nc.scalar.activation(rqk, ssq, AF.Sqrt, bias=eps_sb,
                                         scale=1.0 / HEAD_DIM)
                    nc.vector.reciprocal(rqk, rqk)
                    nc.vector.tensor_scalar_mul(q_r[:, c, :], qkv_ps[:, :P],
                                                rqk[:, 0:1])
                    nc.vector.tensor_scalar_mul(k_r[:, c, :], qkv_ps[:, P:2 * P],
                                                rqk[:, 1:2])
                    nc.vector.tensor_tensor(v_r[:, c, :], qkv_ps[:, 2 * P:],
                                            ve_sb[:, c, :], AluOpType.add)

                # RoPE for this block's 4 chunks (in place on q_r/k_r)
                csl = slice(4 * b, 4 * b + 4)
                for t_r in (q_r, k_r):
                    t1 = rope.tile([P, 4, 32], F32, tag="t1", name="t1")
                    t2 = rope.tile([P, 4, 32], F32, tag="t2", name="t2")
                    t4 = rope.tile([P, 4, 32], F32, tag="t4", name="t4")
                    t5 = rope.tile([P, 4, 32], F32, tag="t5", name="t5")
                    x1 = t_r[:, csl, 0:32]
                    x2 = t_r[:, csl, 64:96]
                    ca = cos_sb[:, csl, 0:32]
                    sa = sin_sb[:, csl, 0:32]
                    nc.vector.tensor_tensor(t1, x1, ca, AluOpType.mult)
                    nc.vector.tensor_tensor(t2, x2, sa, AluOpType.mult)
                    nc.vector.tensor_tensor(t4, x1, sa, AluOpType.mult)
                    nc.vector.tensor_tensor(t5, x2, ca, AluOpType.mult)
                    nc.vector.tensor_tensor(x1, t1, t2, AluOpType.add)
                    nc.vector.tensor_tensor(x2, t5, t4, AluOpType.subtract)

                # transposes for this block (evac split ACT/DVE)
                for c in range(4 * b, 4 * b + 4):
                    for j, (src_t, dstT) in enumerate(((q_r, qT_sb), (k_r, kT_sb))):
                        tp = ps_tr.tile([P, P], F32R, tag="tp", name="tp")
                        nc.tensor.transpose(tp, src_t[:, c, :], id_sb)
                        if j == 0:
                            nc.scalar.copy(dstT[:, c * P:(c + 1) * P], tp)
                        else:
                            nc.vector.tensor_copy(dstT[:, c * P:(c + 1) * P], tp)

            # ---- phase 2: attention + pipelined c_proj + ReduceScatter ----
            yT_sb = big.tile([P, T], F32R)

            def _emit_cproj(j):
                jsl = slice(j * 512, (j + 1) * 512)
                for o in range(ND):
                    cp_ps = ps_mm.tile([P, 512], F32, tag="mm", name="cp_ps")
                    nc.tensor.matmul(cp_ps, cpw_sb[:, o * P:(o + 1) * P],
                                     yT_sb[:, jsl], start=True, stop=True)
                    ev = work.tile([P, 512], F32, tag="ev", name="ev")
                    if o % 2 == 0:
                        nc.vector.tensor_copy(ev, cp_ps)
                    else:
                        nc.scalar.copy(ev, cp_ps)
                    nc.sync.dma_start(partials[j][o * P:(o + 1) * P, :], ev)
                if with_rs:
                    nc.gpsimd.collective_compute(
                        "ReduceScatter",
                        mybir.AluOpType.add,
                        replica_groups=[list(range(NCORES))],
                        ins=[partials[j][:].opt()],
                        outs=[rs_outs[j][:].opt()],
                    )
                    res = work.tile([P, 512], F32, tag="res", name="res")
                    nc.sync.dma_start(res, rs_outs[j][:, :])
                    nc.sync.dma_start(out_s[:, jsl], res)
            def _attn_block(i):
                tsl = slice(i * 512, (i + 1) * 512)
                n_s = 4 * i + 4
                yT_ps = ps_acc.tile([P, 512], F32, tag="yT", name="yT_ps")
                den_ps = ps_den.tile([1, 512], F32, tag="den", name="den_ps")
                s_order = list(range(4 * i, n_s)) + list(range(0, 4 * i))
                for si, s in enumerate(s_order):
                    lo = s - 4 * i            # staircase chunk idx when >= 0
                    m0 = 0 if lo < 0 else min(lo, 2) * P   # matmul col start
                    e0 = 0 if lo < 0 else lo * P           # exp col start
                    sc_ps = ps_mm.tile([P, 512], F32, tag="mm", name="sc_ps")
                    nc.tensor.matmul(sc_ps[:, m0:], kT_sb[:, s * P:(s + 1) * P],
                                     qT_sb[:, i * 512 + m0:(i + 1) * 512],
                                     start=True, stop=True)
                    pT = work.tile([P, 512], F32R, tag="pT", name="pT")
                    nc.scalar.activation(pT[:, e0:], sc_ps[:, e0:], AF.Exp,
                                         scale=ATTN_SCALE)
                    if lo > 0:
                        nc.gpsimd.tensor_copy(pT[:, :e0], zeros_sb[:, :e0])
                    if lo >= 0:
                        nc.vector.tensor_tensor(
                            pT[:, e0:e0 + P], pT[:, e0:e0 + P], tri_sb,
                            AluOpType.mult)
                    nc.tensor.matmul(yT_ps[:, m0:], v_r[:, s, :], pT[:, m0:],
                                     start=(si == 0), stop=(si == n_s - 1))
                    nc.tensor.matmul(den_ps[:, m0:], ones_r, pT[:, m0:],
                                     start=(si == 0), stop=(si == n_s - 1))
                # denominator: reciprocal from PSUM, gpsimd partition bcast
                den_sb = small.tile([1, 512], F32, tag="densb", name="den_sb")
                nc.vector.reciprocal(den_sb, den_ps)
                rec_sb = work.tile([P, 512], F32, tag="rec", name="rec_sb")
                nc.gpsimd.partition_broadcast(rec_sb, den_sb)
                nc.vector.tensor_tensor(yT_sb[:, tsl], yT_ps, rec_sb,
                                        AluOpType.mult)

            # ---- phase 1 then attention (keeps ACT table sets stable) ----
            for b in range(NB):
                _phase1_block(b)
            for b in range(NB):
                _attn_block(b)
                if b > 0:
                    _emit_cproj(b - 1)
            _emit_cproj(NB - 1)

